# revision 22
# baseline (speedup 1.0000x reference)
import os
import sys

import numpy as np
import ml_dtypes

if "/opt/trn_rl_repo" not in sys.path:
    sys.path.insert(0, "/opt/trn_rl_repo")

import concourse.bass as bass
import concourse.mybir as mybir
import concourse.tile as tile
from concourse import bacc
from concourse.bass_utils import run_bass_kernel_spmd

P = 128
B, N, E = 64, 10000, 320000
LAMBDA_PHY = 0.3
NCORES = 8
EPC = E // NCORES              # 40000 real edges per core

# quad/chunk geometry (per core): ten 8-qslot chunks + one 2-qslot chunk
CHUNK_QS = [8] * 10 + [2]      # qslots per compute chunk
NCHUNK = len(CHUNK_QS)
QSLOTS = sum(CHUNK_QS)         # 82 quad slots
QPAD = QSLOTS * P              # 10496 quads
EPADC = QPAD * 4               # 41984 edge slots
ESLOTS = EPADC // P            # 328 edge slots
QS_BASE = [0]
for _cs in CHUNK_QS:
    QS_BASE.append(QS_BASE[-1] + _cs)

# >1024 idxs per dma_gather call crashes the device (ucode cap)
SRC_CALL = int(os.environ.get("K_SRC_CALL", "1024"))   # idxs per src dma_gather
DST_CALL = int(os.environ.get("K_DST_CALL", "1024"))   # idxs per dst dma_gather
SCRATCH = int(os.environ.get("K_SCRATCH", "49152"))

NDL = N // NCORES              # 1250 data-loss columns per core
DL_F = B * NDL // P            # 625

FP = mybir.dt.float32
BF = mybir.dt.bfloat16
I16 = mybir.dt.int16

LAST_EXEC_NS = None
LAST_PROFILE = None

_NC_CACHE = {}


def _build_nc():
    if "nc" in _NC_CACHE:
        return _NC_CACHE["nc"]
    nc = bacc.Bacc(
        None,
        target_bir_lowering=False,
        num_swdge_queues=4,
        dynamic_dma_scratch_size=SCRATCH,
    )

    table_d = nc.declare_dram_parameter("table", [N, P], BF, isOutput=False)
    sidx_d = nc.declare_dram_parameter("sidx", [P, EPADC // 16], I16, isOutput=False)
    oneh_d = nc.declare_dram_parameter("oneh", [P, QSLOTS, 2, P], BF, isOutput=False)
    win_d = nc.declare_dram_parameter("win", [P, QSLOTS, 2, P], BF, isOutput=False)
    c0_d = nc.declare_dram_parameter("c0a", [P, ESLOTS], BF, isOutput=False)
    c1_d = nc.declare_dram_parameter("c1a", [P, ESLOTS], BF, isOutput=False)
    c2_d = nc.declare_dram_parameter("c2a", [P, ESLOTS], BF, isOutput=False)
    pdl_d = nc.declare_dram_parameter("pdl", [P, DL_F], FP, isOutput=False)
    tdl_d = nc.declare_dram_parameter("tdl", [P, DL_F], FP, isOutput=False)
    out_d = nc.declare_dram_parameter("partials", [P, 2], FP, isOutput=True)

    with tile.TileContext(nc) as tc:
        with tc.tile_pool(name="sbuf", bufs=1) as pool, tc.tile_pool(
            name="psum", bufs=1, space="PSUM"
        ) as ppool:
            sidx_t = pool.tile([P, EPADC // 16], I16)
            c0_t = pool.tile([P, ESLOTS], BF)
            c1_t = pool.tile([P, ESLOTS], BF)
            c2_t = pool.tile([P, ESLOTS], BF)
            pdl_t = pool.tile([P, DL_F], FP)
            tdl_t = pool.tile([P, DL_F], FP)
            dd_t = pool.tile([P, DL_F], FP)
            dacc = pool.tile([P, 1], FP)
            phy_acc = pool.tile([P, 1], FP)
            chunk_accs = pool.tile([P, NCHUNK], FP)

            NBUF = 7
            gs_t = [pool.tile([P, 32, P], BF, name=f"gs{i}") for i in range(NBUF)]
            qd_t = [pool.tile([P, 8, P], BF, name=f"qd{i}") for i in range(2)]
            oneh_t = [pool.tile([P, 8, 2, P], BF, name=f"oneh{i}") for i in range(3)]
            win_t = [pool.tile([P, 8, 2, P], BF, name=f"win{i}") for i in range(3)]
            qd_ps = [ppool.tile([P, 8, P], FP, name=f"qdps{i}") for i in range(3)]
            m0_t = [pool.tile([P, 32, B], BF, name=f"m0_{i}") for i in range(2)]
            m1_t = [pool.tile([P, 32, B], BF, name=f"m1_{i}") for i in range(2)]
            u_t = [pool.tile([P, 32, B], BF, name=f"u{i}") for i in range(2)]
            m2_t = [pool.tile([P, 8, B], BF, name=f"m2_{i}") for i in range(2)]
            r_t = [pool.tile([P, 32, B], BF, name=f"r{i}") for i in range(2)]
            c0e_t = [pool.tile([P, 32, B], BF, name=f"c0e{i}") for i in range(2)]
            c1e_t = [pool.tile([P, 32, B], BF, name=f"c1e{i}") for i in range(2)]

            # load the first two chunks' gather idxs first so gathers can
            # start early; bulk idxs and data-loss arrays follow
            nc.sync.dma_start(out=sidx_t[:, 0:512], in_=sidx_d[:, 0:512])
            nc.sync.dma_start(out=c0_t[:], in_=c0_d[:])
            nc.sync.dma_start(out=c1_t[:], in_=c1_d[:])
            nc.sync.dma_start(out=c2_t[:], in_=c2_d[:])
            nc.sync.dma_start(
                out=sidx_t[:, 512 : EPADC // 16], in_=sidx_d[:, 512 : EPADC // 16]
            )
            nc.sync.dma_start(out=pdl_t[:], in_=pdl_d[:])
            nc.sync.dma_start(out=tdl_t[:], in_=tdl_d[:])

            # data loss partial: sum((pred - target)^2), square+reduce on Act
            nc.vector.tensor_tensor(
                out=dd_t[:], in0=pdl_t[:], in1=tdl_t[:], op=mybir.AluOpType.subtract
            )
            nc.scalar.activation(
                out=pdl_t[:],
                in_=dd_t[:],
                func=mybir.ActivationFunctionType.Square,
                accum_out=dacc[:],
            )

            mul = mybir.AluOpType.mult
            sub = mybir.AluOpType.subtract
            add = mybir.AluOpType.add

            qn = [0]  # round-robin queue counter for gathers
            for k in range(NCHUNK):
                cs = CHUNK_QS[k]
                qs0 = QS_BASE[k]
                es = 4 * cs
                gs = gs_t[k % NBUF]
                qd = qd_t[k % 2]
                oh = oneh_t[k % 3]
                wn = win_t[k % 3]
                qp = qd_ps[k % 3]
                m0 = m0_t[k % 2]
                m1 = m1_t[k % 2]
                u = u_t[k % 2]
                m2 = m2_t[k % 2]
                r = r_t[k % 2]
                c0e = c0e_t[k % 2]
                c1e = c1e_t[k % 2]

                # src gather: 512*cs idxs for this chunk (split into calls)
                base = qs0 * 512
                for ci in range(0, 512 * cs, SRC_CALL):
                    i0 = base + ci
                    nc.gpsimd.dma_gather(
                        out_ap=gs[:, ci // 128 : (ci + SRC_CALL) // 128, :],
                        in_ap=table_d[:, :],
                        idxs_ap=sidx_t[:, i0 // 16 : (i0 + SRC_CALL) // 16],
                        num_idxs=SRC_CALL,
                        num_idxs_reg=SRC_CALL,
                        elem_size=P,
                        queue_num=qn[0] % 4,
                    )
                    qn[0] += 1
                # dst expansion via PE one-hot matmuls (no dma_gather):
                nc.sync.dma_start(
                    out=oh[:, 0:cs, :, :], in_=oneh_d[:, qs0 : qs0 + cs, :, :]
                )
                nc.sync.dma_start(
                    out=wn[:, 0:cs, :, :], in_=win_d[:, qs0 : qs0 + cs, :, :]
                )

                so = qs0 * 4
                c0b = c0_t[:, so : so + es, None].to_broadcast([P, es, B])
                c1b = c1_t[:, so : so + es, None].to_broadcast([P, es, B])
                # expand coefficients dense on Act so the DVE mults hit 2x mode
                nc.scalar.copy(out=c0e[:, 0:es, :], in_=c0b)
                nc.scalar.copy(out=c1e[:, 0:es, :], in_=c1b)
                for qq in range(cs):
                    for h in range(2):
                        nc.tensor.matmul(
                            qp[:, qq, :],
                            oh[:, qq, h, :],
                            wn[:, qq, h, :],
                            start=(h == 0),
                            stop=(h == 1),
                        )
                nc.scalar.copy(out=qd[:, 0:cs, :], in_=qp[:, 0:cs, :])
                # u = c0*ps + c1*prs
                nc.vector.tensor_tensor(
                    out=m0[:, 0:es, :], in0=gs[:, 0:es, 0:B], in1=c0e[:, 0:es, :], op=mul
                )
                nc.vector.tensor_tensor(
                    out=m1[:, 0:es, :], in0=gs[:, 0:es, B:P], in1=c1e[:, 0:es, :], op=mul
                )
                nc.vector.tensor_tensor(
                    out=u[:, 0:es, :], in0=m0[:, 0:es, :], in1=m1[:, 0:es, :], op=add
                )
                # per j: m2 = c2*prd ; r_j = (pd - m2) - u_j
                for j in range(4):
                    sl = slice(cs * j, cs * j + cs)
                    c2bj = c2_t[
                        :, so + cs * j : so + cs * j + cs, None
                    ].to_broadcast([P, cs, B])
                    nc.vector.tensor_tensor(
                        out=m2[:, 0:cs, :], in0=qd[:, 0:cs, B:P], in1=c2bj, op=mul
                    )
                    nc.vector.tensor_tensor(
                        out=r[:, sl, :], in0=qd[:, 0:cs, 0:B], in1=m2[:, 0:cs, :], op=sub
                    )
                    nc.vector.tensor_tensor(
                        out=r[:, sl, :], in0=r[:, sl, :], in1=u[:, sl, :], op=sub
                    )
                # Act: square + accumulate -> chunk_accs[:, k]
                nc.scalar.activation(
                    out=m0[:, 0:es, :],
                    in_=r[:, 0:es, :],
                    func=mybir.ActivationFunctionType.Square,
                    accum_out=chunk_accs[:, k : k + 1],
                )

            nc.vector.tensor_reduce(
                out=phy_acc[:],
                in_=chunk_accs[:],
                axis=mybir.AxisListType.X,
                op=mybir.AluOpType.add,
            )
            nc.sync.dma_start(out=out_d[:, 0:1], in_=phy_acc[:])
            nc.sync.dma_start(out=out_d[:, 1:2], in_=dacc[:])

    nc.finalize()
    _NC_CACHE["nc"] = nc
    return nc


def _wrap_idx(idx: np.ndarray) -> np.ndarray:
    # dma_gather layout: index i lives at partition i%16, column i//16,
    # replicated across the 8 groups of 16 partitions
    n = idx.shape[0]
    w16 = idx.reshape(n // 16, 16).T
    return np.ascontiguousarray(np.tile(w16, (8, 1)))


def _prep_core(s, d, c0, c1, c2):
    """Build one core's padded quad-major edge arrays.

    Edges arrive sorted by dst. Each dst run is padded to a multiple of 4
    with synthetic edges (src=dst, c0=1, c1=c2=0) whose residual is exactly
    zero. Leftover quad slots are filled with node-0 synthetic edges.
    Returns (src_seq, dst_quad, c0_seq, c1_seq, c2_seq) where the _seq
    arrays are in gather-position order (length EPADC) and dst_quad has
    one entry per quad (length QPAD).
    """
    uds, counts = np.unique(d, return_counts=True)
    pad_counts = (-counts) % 4
    padded = counts + pad_counts
    tot = int(padded.sum())
    assert tot <= EPADC, f"padded edges {tot} > {EPADC}"

    starts = np.concatenate(([0], np.cumsum(padded)))[:-1]
    run_starts = np.concatenate(([0], np.cumsum(counts)))[:-1]
    pos = np.repeat(starts, counts) + (np.arange(len(d)) - np.repeat(run_starts, counts))

    dst_p = np.zeros(EPADC, np.int64)
    dst_p[:tot] = np.repeat(uds, padded)
    if tot < EPADC:
        dst_p[tot:] = dst_p[tot - 1]  # full-pad quads reuse last dst
    src_p = dst_p.copy()              # synthetic edges: src = dst
    c0_p = np.ones(EPADC, np.float32)  # synthetic: c0=1 -> r = pd - pd = 0
    c1_p = np.zeros(EPADC, np.float32)
    c2_p = np.zeros(EPADC, np.float32)
    src_p[pos] = s
    c0_p[pos] = c0
    c1_p[pos] = c1
    c2_p[pos] = c2

    # quad dst index (one per quad; all 4 edges of a quad share dst)
    dst_quad = dst_p[0::4]

    # edge (q, j) -> gather position (k*32 + j*8 + qq)*128 + p
    e = np.arange(EPADC)
    q, j = e >> 2, e & 3
    qs_of_q = q // P                     # global qslot of quad
    bounds = np.array(QS_BASE[1:])
    k = np.searchsorted(bounds, qs_of_q, side="right")
    qs0 = np.array(QS_BASE)[k]
    cs = np.array(CHUNK_QS)[k]
    qq = qs_of_q - qs0
    p = q % P
    gpos = ((qs0 * 4) + j * cs + qq) * P + p

    src_seq = np.empty(EPADC, np.int16)
    src_seq[gpos] = src_p.astype(np.int16)
    c0_seq = np.empty(EPADC, np.float32)
    c0_seq[gpos] = c0_p
    c1_seq = np.empty(EPADC, np.float32)
    c1_seq[gpos] = c1_p
    c2_seq = np.empty(EPADC, np.float32)
    c2_seq[gpos] = c2_p
    return src_seq, dst_quad, c0_seq, c1_seq, c2_seq


def _onehot_windows(dst_quad, table_bf):
    """Per qslot: 2 (onehot, window) operand pairs for the PE dst expansion."""
    NBLKT = (N + P - 1) // P
    tpad = np.zeros((NBLKT * P, P), ml_dtypes.bfloat16)
    tpad[:N] = table_bf
    oneh = np.zeros((P, QSLOTS, 2, P), np.float32)
    win = np.empty((P, QSLOTS, 2, P), ml_dtypes.bfloat16)
    dq = dst_quad.reshape(QSLOTS, P)
    blkA = (dq[:, 0] // P).astype(np.int64)
    blkB = np.minimum(blkA + 1, NBLKT - 1)
    h_of = (dq // P) - blkA[:, None]
    assert h_of.min() >= 0 and h_of.max() <= 1, "qslot spans >2 blocks"
    s_idx = np.repeat(np.arange(QSLOTS), P)
    m_idx = np.tile(np.arange(P), QSLOTS)
    oneh[(dq % P).ravel(), s_idx, h_of.ravel(), m_idx] = 1.0
    tb = tpad.reshape(NBLKT, P, P)
    win[:, :, 0, :] = tb[blkA].transpose(1, 0, 2)
    win[:, :, 1, :] = tb[blkB].transpose(1, 0, 2)
    return (
        np.ascontiguousarray(oneh.astype(ml_dtypes.bfloat16)),
        np.ascontiguousarray(win),
    )


def _coeff_tile(seq: np.ndarray) -> np.ndarray:
    # gather-position order -> [P, ESLOTS] (pos = col*128 + p)
    return np.ascontiguousarray(
        seq.reshape(ESLOTS, P).T.astype(ml_dtypes.bfloat16)
    )


def kernel(**inputs) -> np.ndarray:
    global LAST_EXEC_NS, LAST_PROFILE
    pred = np.ascontiguousarray(np.asarray(inputs["pred"], dtype=np.float32))
    target = np.ascontiguousarray(np.asarray(inputs["target"], dtype=np.float32))
    prev_target = np.ascontiguousarray(
        np.asarray(inputs["prev_target"], dtype=np.float32)
    )
    c0 = np.asarray(inputs["c0"], dtype=np.float32)
    c1 = np.asarray(inputs["c1"], dtype=np.float32)
    c2 = np.asarray(inputs["c2"], dtype=np.float32)
    edge_index = np.asarray(inputs["edge_index"])
    src = edge_index[0].astype(np.int64)
    dst = edge_index[1].astype(np.int64)

    # sort edges by (dst, src); contiguous 1/8 chunks per core
    order = np.lexsort((src, dst))
    src_s, dst_s = src[order], dst[order]
    c0_s, c1_s, c2_s = c0[order], c1[order], c2[order]

    # gather table: row n = [pred[:, n] | prev_target[:, n]] in bf16 (256B)
    table = np.ascontiguousarray(
        np.concatenate([pred.T, prev_target.T], axis=1).astype(ml_dtypes.bfloat16)
    )

    in_maps = []
    for c in range(NCORES):
        esl = slice(c * EPC, (c + 1) * EPC)
        src_seq, dst_quad, c0_seq, c1_seq, c2_seq = _prep_core(
            src_s[esl], dst_s[esl], c0_s[esl], c1_s[esl], c2_s[esl]
        )
        oneh, win = _onehot_windows(dst_quad, table)
        nsl = slice(c * NDL, (c + 1) * NDL)
        in_maps.append(
            {
                "table": table,
                "sidx": _wrap_idx(src_seq),
                "oneh": oneh,
                "win": win,
                "c0a": _coeff_tile(c0_seq),
                "c1a": _coeff_tile(c1_seq),
                "c2a": _coeff_tile(c2_seq),
                "pdl": np.ascontiguousarray(pred[:, nsl].reshape(P, DL_F)),
                "tdl": np.ascontiguousarray(target[:, nsl].reshape(P, DL_F)),
            }
        )

    nc = _build_nc()
    res = run_bass_kernel_spmd(nc, in_maps, list(range(NCORES)))
    LAST_EXEC_NS = res.exec_time_ns
    LAST_PROFILE = res.profile_json

    phy_sum = 0.0
    data_sum = 0.0
    for c in range(NCORES):
        part = np.asarray(res.results[c]["partials"], dtype=np.float64)
        phy_sum += part[:, 0].sum()
        data_sum += part[:, 1].sum()

    data_loss = data_sum / (B * N)
    phy_loss = phy_sum / (B * E)
    total = data_loss + LAMBDA_PHY * phy_loss
    return np.array([total, data_loss, phy_loss], dtype=np.float32)


if __name__ == "__main__":
    rng = np.random.default_rng(0)
    ins = {
        "pred": rng.standard_normal((B, N), dtype=np.float32),
        "target": rng.standard_normal((B, N), dtype=np.float32),
        "prev_target": rng.standard_normal((B, N), dtype=np.float32),
        "c0": rng.random(E, dtype=np.float32),
        "c1": rng.random(E, dtype=np.float32),
        "c2": rng.random(E, dtype=np.float32),
        "edge_index": rng.integers(0, N, (2, E)).astype(np.int64),
    }
    out = kernel(**ins)
    # numpy check
    p64 = ins["pred"].astype(np.float64)
    t64 = ins["target"].astype(np.float64)
    pv64 = ins["prev_target"].astype(np.float64)
    s, d = ins["edge_index"]
    dl = np.mean((p64 - t64) ** 2)
    exp = (ins["c0"] * p64[:, s] + ins["c1"] * pv64[:, s] + ins["c2"] * pv64[:, d])
    res_ = p64[:, d] - exp
    pl = np.mean(res_ ** 2)
    ref = np.array([dl + LAMBDA_PHY * pl, dl, pl])
    print("kernel out:", out)
    print("numpy ref :", ref)
    print("rel err   :", np.abs(out - ref) / np.abs(ref))


# revision 23
# speedup vs baseline: 1.0671x; 1.0671x over previous
import os
import sys

import numpy as np
import ml_dtypes

if "/opt/trn_rl_repo" not in sys.path:
    sys.path.insert(0, "/opt/trn_rl_repo")

import concourse.bass as bass
import concourse.mybir as mybir
import concourse.tile as tile
from concourse import bacc
from concourse.bass_utils import run_bass_kernel_spmd

P = 128
B, N, E = 64, 10000, 320000
LAMBDA_PHY = 0.3
NCORES = 8
EPC = E // NCORES              # 40000 real edges per core

# quad/chunk geometry (per core): ten 8-qslot chunks + one 2-qslot chunk
CHUNK_QS = [8] * 10 + [2]      # qslots per compute chunk
NCHUNK = len(CHUNK_QS)
QSLOTS = sum(CHUNK_QS)         # 82 quad slots
QPAD = QSLOTS * P              # 10496 quads
EPADC = QPAD * 4               # 41984 edge slots
ESLOTS = EPADC // P            # 328 edge slots
QS_BASE = [0]
for _cs in CHUNK_QS:
    QS_BASE.append(QS_BASE[-1] + _cs)

# >1024 idxs per dma_gather call crashes the device (ucode cap)
SRC_CALL = int(os.environ.get("K_SRC_CALL", "1024"))   # idxs per src dma_gather
DST_CALL = int(os.environ.get("K_DST_CALL", "1024"))   # idxs per dst dma_gather
SCRATCH = int(os.environ.get("K_SCRATCH", "49152"))

NDL = N // NCORES              # 1250 data-loss columns per core
DL_F = B * NDL // P            # 625

FP = mybir.dt.float32
BF = mybir.dt.bfloat16
I16 = mybir.dt.int16

LAST_EXEC_NS = None
LAST_PROFILE = None

_NC_CACHE = {}


def _build_nc():
    if "nc" in _NC_CACHE:
        return _NC_CACHE["nc"]
    nc = bacc.Bacc(
        None,
        target_bir_lowering=False,
        num_swdge_queues=4,
        dynamic_dma_scratch_size=SCRATCH,
    )

    table_d = nc.declare_dram_parameter("table", [N, P], BF, isOutput=False)
    sidx_d = nc.declare_dram_parameter("sidx", [P, EPADC // 16], I16, isOutput=False)
    oneh_d = nc.declare_dram_parameter("oneh", [P, QSLOTS, 2, P], BF, isOutput=False)
    win_d = nc.declare_dram_parameter("win", [P, QSLOTS, 2, P], BF, isOutput=False)
    c0_d = nc.declare_dram_parameter("c0a", [P, ESLOTS], BF, isOutput=False)
    c1_d = nc.declare_dram_parameter("c1a", [P, ESLOTS], BF, isOutput=False)
    c2_d = nc.declare_dram_parameter("c2a", [P, ESLOTS], BF, isOutput=False)
    pdl_d = nc.declare_dram_parameter("pdl", [P, DL_F], FP, isOutput=False)
    tdl_d = nc.declare_dram_parameter("tdl", [P, DL_F], FP, isOutput=False)
    out_d = nc.declare_dram_parameter("partials", [P, 2], FP, isOutput=True)

    with tile.TileContext(nc) as tc:
        with tc.tile_pool(name="sbuf", bufs=1) as pool, tc.tile_pool(
            name="psum", bufs=1, space="PSUM"
        ) as ppool:
            sidx_t = pool.tile([P, EPADC // 16], I16)
            c0_t = pool.tile([P, ESLOTS], BF)
            c1_t = pool.tile([P, ESLOTS], BF)
            c2_t = pool.tile([P, ESLOTS], BF)
            pdl_t = pool.tile([P, DL_F], FP)
            tdl_t = pool.tile([P, DL_F], FP)
            dd_t = pool.tile([P, DL_F], FP)
            dacc = pool.tile([P, 1], FP)
            phy_acc = pool.tile([P, 1], FP)
            chunk_accs = pool.tile([P, NCHUNK], FP)

            NBUF = 7
            gs_t = [pool.tile([P, 32, P], BF, name=f"gs{i}") for i in range(NBUF)]
            qd_t = [pool.tile([P, 8, P], BF, name=f"qd{i}") for i in range(2)]
            oneh_t = [pool.tile([P, 8, 2, P], BF, name=f"oneh{i}") for i in range(3)]
            win_t = [pool.tile([P, 8, 2, P], BF, name=f"win{i}") for i in range(3)]
            qd_ps = [ppool.tile([P, 8, P], FP, name=f"qdps{i}") for i in range(3)]
            m0_t = [pool.tile([P, 32, B], BF, name=f"m0_{i}") for i in range(2)]
            m1_t = [pool.tile([P, 32, B], BF, name=f"m1_{i}") for i in range(2)]
            u_t = [pool.tile([P, 32, B], BF, name=f"u{i}") for i in range(2)]
            m2_t = [pool.tile([P, 8, B], BF, name=f"m2_{i}") for i in range(2)]
            r_t = [pool.tile([P, 32, B], BF, name=f"r{i}") for i in range(2)]
            c0e_t = [pool.tile([P, 32, B], BF, name=f"c0e{i}") for i in range(2)]
            c1e_t = [pool.tile([P, 32, B], BF, name=f"c1e{i}") for i in range(2)]

            nc.sync.dma_start(out=sidx_t[:], in_=sidx_d[:])
            nc.sync.dma_start(out=c0_t[:], in_=c0_d[:])
            nc.sync.dma_start(out=c1_t[:], in_=c1_d[:])
            nc.sync.dma_start(out=c2_t[:], in_=c2_d[:])
            nc.sync.dma_start(out=pdl_t[:], in_=pdl_d[:])
            nc.sync.dma_start(out=tdl_t[:], in_=tdl_d[:])

            # data loss partial: sum((pred - target)^2), square+reduce on Act
            nc.vector.tensor_tensor(
                out=dd_t[:], in0=pdl_t[:], in1=tdl_t[:], op=mybir.AluOpType.subtract
            )
            nc.scalar.activation(
                out=pdl_t[:],
                in_=dd_t[:],
                func=mybir.ActivationFunctionType.Square,
                accum_out=dacc[:],
            )

            mul = mybir.AluOpType.mult
            sub = mybir.AluOpType.subtract
            add = mybir.AluOpType.add

            qn = [0]  # round-robin queue counter for gathers
            for k in range(NCHUNK):
                cs = CHUNK_QS[k]
                qs0 = QS_BASE[k]
                es = 4 * cs
                gs = gs_t[k % NBUF]
                qd = qd_t[k % 2]
                oh = oneh_t[k % 3]
                wn = win_t[k % 3]
                qp = qd_ps[k % 3]
                m0 = m0_t[k % 2]
                m1 = m1_t[k % 2]
                u = u_t[k % 2]
                m2 = m2_t[k % 2]
                r = r_t[k % 2]
                c0e = c0e_t[k % 2]
                c1e = c1e_t[k % 2]

                # src gather: 512*cs idxs for this chunk (split into calls)
                base = qs0 * 512
                for ci in range(0, 512 * cs, SRC_CALL):
                    i0 = base + ci
                    nc.gpsimd.dma_gather(
                        out_ap=gs[:, ci // 128 : (ci + SRC_CALL) // 128, :],
                        in_ap=table_d[:, :],
                        idxs_ap=sidx_t[:, i0 // 16 : (i0 + SRC_CALL) // 16],
                        num_idxs=SRC_CALL,
                        num_idxs_reg=SRC_CALL,
                        elem_size=P,
                        queue_num=qn[0] % 4,
                    )
                    qn[0] += 1
                # dst expansion via PE one-hot matmuls (no dma_gather):
                nc.sync.dma_start(
                    out=oh[:, 0:cs, :, :], in_=oneh_d[:, qs0 : qs0 + cs, :, :]
                )
                nc.sync.dma_start(
                    out=wn[:, 0:cs, :, :], in_=win_d[:, qs0 : qs0 + cs, :, :]
                )

                so = qs0 * 4
                c0b = c0_t[:, so : so + es, None].to_broadcast([P, es, B])
                c1b = c1_t[:, so : so + es, None].to_broadcast([P, es, B])
                # expand coefficients dense on Act so the DVE mults hit 2x mode
                nc.scalar.copy(out=c0e[:, 0:es, :], in_=c0b)
                nc.scalar.copy(out=c1e[:, 0:es, :], in_=c1b)
                for qq in range(cs):
                    for h in range(2):
                        nc.tensor.matmul(
                            qp[:, qq, :],
                            oh[:, qq, h, :],
                            wn[:, qq, h, :],
                            start=(h == 0),
                            stop=(h == 1),
                        )
                nc.scalar.copy(out=qd[:, 0:cs, :], in_=qp[:, 0:cs, :])
                # u = c0*ps + c1*prs
                nc.vector.tensor_tensor(
                    out=m0[:, 0:es, :], in0=gs[:, 0:es, 0:B], in1=c0e[:, 0:es, :], op=mul
                )
                nc.vector.tensor_tensor(
                    out=m1[:, 0:es, :], in0=gs[:, 0:es, B:P], in1=c1e[:, 0:es, :], op=mul
                )
                nc.vector.tensor_tensor(
                    out=u[:, 0:es, :], in0=m0[:, 0:es, :], in1=m1[:, 0:es, :], op=add
                )
                # per j: m2 = c2*prd ; r_j = (pd - m2) - u_j
                for j in range(4):
                    sl = slice(cs * j, cs * j + cs)
                    c2bj = c2_t[
                        :, so + cs * j : so + cs * j + cs, None
                    ].to_broadcast([P, cs, B])
                    nc.vector.tensor_tensor(
                        out=m2[:, 0:cs, :], in0=qd[:, 0:cs, B:P], in1=c2bj, op=mul
                    )
                    nc.vector.tensor_tensor(
                        out=r[:, sl, :], in0=qd[:, 0:cs, 0:B], in1=m2[:, 0:cs, :], op=sub
                    )
                    nc.vector.tensor_tensor(
                        out=r[:, sl, :], in0=r[:, sl, :], in1=u[:, sl, :], op=sub
                    )
                # Act: square + accumulate -> chunk_accs[:, k]
                nc.scalar.activation(
                    out=m0[:, 0:es, :],
                    in_=r[:, 0:es, :],
                    func=mybir.ActivationFunctionType.Square,
                    accum_out=chunk_accs[:, k : k + 1],
                )

            nc.vector.tensor_reduce(
                out=phy_acc[:],
                in_=chunk_accs[:],
                axis=mybir.AxisListType.X,
                op=mybir.AluOpType.add,
            )
            nc.sync.dma_start(out=out_d[:, 0:1], in_=phy_acc[:])
            nc.sync.dma_start(out=out_d[:, 1:2], in_=dacc[:])

    nc.finalize()
    _NC_CACHE["nc"] = nc
    return nc


def _wrap_idx(idx: np.ndarray) -> np.ndarray:
    # dma_gather layout: index i lives at partition i%16, column i//16,
    # replicated across the 8 groups of 16 partitions
    n = idx.shape[0]
    w16 = idx.reshape(n // 16, 16).T
    return np.ascontiguousarray(np.tile(w16, (8, 1)))


def _prep_core(s, d, c0, c1, c2):
    """Build one core's padded quad-major edge arrays.

    Edges arrive sorted by dst. Each dst run is padded to a multiple of 4
    with synthetic edges (src=dst, c0=1, c1=c2=0) whose residual is exactly
    zero. Leftover quad slots are filled with node-0 synthetic edges.
    Returns (src_seq, dst_quad, c0_seq, c1_seq, c2_seq) where the _seq
    arrays are in gather-position order (length EPADC) and dst_quad has
    one entry per quad (length QPAD).
    """
    uds, counts = np.unique(d, return_counts=True)
    pad_counts = (-counts) % 4
    padded = counts + pad_counts
    tot = int(padded.sum())
    assert tot <= EPADC, f"padded edges {tot} > {EPADC}"

    starts = np.concatenate(([0], np.cumsum(padded)))[:-1]
    run_starts = np.concatenate(([0], np.cumsum(counts)))[:-1]
    pos = np.repeat(starts, counts) + (np.arange(len(d)) - np.repeat(run_starts, counts))

    dst_p = np.zeros(EPADC, np.int64)
    dst_p[:tot] = np.repeat(uds, padded)
    if tot < EPADC:
        dst_p[tot:] = dst_p[tot - 1]  # full-pad quads reuse last dst
    src_p = dst_p.copy()              # synthetic edges: src = dst
    c0_p = np.ones(EPADC, np.float32)  # synthetic: c0=1 -> r = pd - pd = 0
    c1_p = np.zeros(EPADC, np.float32)
    c2_p = np.zeros(EPADC, np.float32)
    src_p[pos] = s
    c0_p[pos] = c0
    c1_p[pos] = c1
    c2_p[pos] = c2

    # quad dst index (one per quad; all 4 edges of a quad share dst)
    dst_quad = dst_p[0::4]

    # edge (q, j) -> gather position (k*32 + j*8 + qq)*128 + p
    e = np.arange(EPADC)
    q, j = e >> 2, e & 3
    qs_of_q = q // P                     # global qslot of quad
    bounds = np.array(QS_BASE[1:])
    k = np.searchsorted(bounds, qs_of_q, side="right")
    qs0 = np.array(QS_BASE)[k]
    cs = np.array(CHUNK_QS)[k]
    qq = qs_of_q - qs0
    p = q % P
    gpos = ((qs0 * 4) + j * cs + qq) * P + p

    src_seq = np.empty(EPADC, np.int16)
    src_seq[gpos] = src_p.astype(np.int16)
    c0_seq = np.empty(EPADC, np.float32)
    c0_seq[gpos] = c0_p
    c1_seq = np.empty(EPADC, np.float32)
    c1_seq[gpos] = c1_p
    c2_seq = np.empty(EPADC, np.float32)
    c2_seq[gpos] = c2_p
    return src_seq, dst_quad, c0_seq, c1_seq, c2_seq


def _onehot_windows(dst_quad, table_bf):
    """Per qslot: 2 (onehot, window) operand pairs for the PE dst expansion."""
    NBLKT = (N + P - 1) // P
    tpad = np.zeros((NBLKT * P, P), ml_dtypes.bfloat16)
    tpad[:N] = table_bf
    oneh = np.zeros((P, QSLOTS, 2, P), np.float32)
    win = np.empty((P, QSLOTS, 2, P), ml_dtypes.bfloat16)
    dq = dst_quad.reshape(QSLOTS, P)
    blkA = (dq[:, 0] // P).astype(np.int64)
    blkB = np.minimum(blkA + 1, NBLKT - 1)
    h_of = (dq // P) - blkA[:, None]
    assert h_of.min() >= 0 and h_of.max() <= 1, "qslot spans >2 blocks"
    s_idx = np.repeat(np.arange(QSLOTS), P)
    m_idx = np.tile(np.arange(P), QSLOTS)
    oneh[(dq % P).ravel(), s_idx, h_of.ravel(), m_idx] = 1.0
    tb = tpad.reshape(NBLKT, P, P)
    win[:, :, 0, :] = tb[blkA].transpose(1, 0, 2)
    win[:, :, 1, :] = tb[blkB].transpose(1, 0, 2)
    return (
        np.ascontiguousarray(oneh.astype(ml_dtypes.bfloat16)),
        np.ascontiguousarray(win),
    )


def _coeff_tile(seq: np.ndarray) -> np.ndarray:
    # gather-position order -> [P, ESLOTS] (pos = col*128 + p)
    return np.ascontiguousarray(
        seq.reshape(ESLOTS, P).T.astype(ml_dtypes.bfloat16)
    )


def kernel(**inputs) -> np.ndarray:
    global LAST_EXEC_NS, LAST_PROFILE
    pred = np.ascontiguousarray(np.asarray(inputs["pred"], dtype=np.float32))
    target = np.ascontiguousarray(np.asarray(inputs["target"], dtype=np.float32))
    prev_target = np.ascontiguousarray(
        np.asarray(inputs["prev_target"], dtype=np.float32)
    )
    c0 = np.asarray(inputs["c0"], dtype=np.float32)
    c1 = np.asarray(inputs["c1"], dtype=np.float32)
    c2 = np.asarray(inputs["c2"], dtype=np.float32)
    edge_index = np.asarray(inputs["edge_index"])
    src = edge_index[0].astype(np.int64)
    dst = edge_index[1].astype(np.int64)

    # sort edges by (dst, src); contiguous 1/8 chunks per core
    order = np.lexsort((src, dst))
    src_s, dst_s = src[order], dst[order]
    c0_s, c1_s, c2_s = c0[order], c1[order], c2[order]

    # gather table: row n = [pred[:, n] | prev_target[:, n]] in bf16 (256B)
    table = np.ascontiguousarray(
        np.concatenate([pred.T, prev_target.T], axis=1).astype(ml_dtypes.bfloat16)
    )

    in_maps = []
    for c in range(NCORES):
        esl = slice(c * EPC, (c + 1) * EPC)
        src_seq, dst_quad, c0_seq, c1_seq, c2_seq = _prep_core(
            src_s[esl], dst_s[esl], c0_s[esl], c1_s[esl], c2_s[esl]
        )
        oneh, win = _onehot_windows(dst_quad, table)
        nsl = slice(c * NDL, (c + 1) * NDL)
        in_maps.append(
            {
                "table": table,
                "sidx": _wrap_idx(src_seq),
                "oneh": oneh,
                "win": win,
                "c0a": _coeff_tile(c0_seq),
                "c1a": _coeff_tile(c1_seq),
                "c2a": _coeff_tile(c2_seq),
                "pdl": np.ascontiguousarray(pred[:, nsl].reshape(P, DL_F)),
                "tdl": np.ascontiguousarray(target[:, nsl].reshape(P, DL_F)),
            }
        )

    nc = _build_nc()
    res = run_bass_kernel_spmd(nc, in_maps, list(range(NCORES)))
    LAST_EXEC_NS = res.exec_time_ns
    LAST_PROFILE = res.profile_json

    phy_sum = 0.0
    data_sum = 0.0
    for c in range(NCORES):
        part = np.asarray(res.results[c]["partials"], dtype=np.float64)
        phy_sum += part[:, 0].sum()
        data_sum += part[:, 1].sum()

    data_loss = data_sum / (B * N)
    phy_loss = phy_sum / (B * E)
    total = data_loss + LAMBDA_PHY * phy_loss
    return np.array([total, data_loss, phy_loss], dtype=np.float32)


if __name__ == "__main__":
    rng = np.random.default_rng(0)
    ins = {
        "pred": rng.standard_normal((B, N), dtype=np.float32),
        "target": rng.standard_normal((B, N), dtype=np.float32),
        "prev_target": rng.standard_normal((B, N), dtype=np.float32),
        "c0": rng.random(E, dtype=np.float32),
        "c1": rng.random(E, dtype=np.float32),
        "c2": rng.random(E, dtype=np.float32),
        "edge_index": rng.integers(0, N, (2, E)).astype(np.int64),
    }
    out = kernel(**ins)
    # numpy check
    p64 = ins["pred"].astype(np.float64)
    t64 = ins["target"].astype(np.float64)
    pv64 = ins["prev_target"].astype(np.float64)
    s, d = ins["edge_index"]
    dl = np.mean((p64 - t64) ** 2)
    exp = (ins["c0"] * p64[:, s] + ins["c1"] * pv64[:, s] + ins["c2"] * pv64[:, d])
    res_ = p64[:, d] - exp
    pl = np.mean(res_ ** 2)
    ref = np.array([dl + LAMBDA_PHY * pl, dl, pl])
    print("kernel out:", out)
    print("numpy ref :", ref)
    print("rel err   :", np.abs(out - ref) / np.abs(ref))
